# revision 2
# baseline (speedup 1.0000x reference)
"""Trainium2 Bass kernel v2 for LongRangeTCN.

Per core (BL=4 batches), per layer, per T-half (TH=2048):
  conv: 3-tap dilated conv as PSUM-accumulated matmuls (per-layer fp32 or f32r),
        Act engine evacuates PSUM->XH adding folded BN bias (xh = 0.5*BN(conv)).
  scan: LIF wavefront on DVE over chunks of LC=32 with H=12 warmup
        (2 scalar_tensor_tensor ops per step, all 4 batches = 256 cols wide).
        The A-trajectory goes to a separate AT tile so XH stays read-only.
  sr:   spike + residual X += (A >= 1) on Pool/GpSimd (TS extract + TT add; the
        95ns Q7 launch amortizes over full-width ops). X is float32r so
        f32r-layer matmuls accept it; fp32 layers read a bitcast-fp32 view.
Pipeline: per-engine program order makes conv(li,h=1) on PE run during
scan(li,h=0) on DVE, Pool's sr(li,h) during scan(li,h+1).
"""

import numpy as np

TAU, VTH, EPS, K = 2.0, 1.0, 1e-5, 3
DILATIONS = (1, 2, 4, 8)
B, C, T = 32, 128, 4096
NCORES = 8
BL = B // NCORES          # 4 batches per core
LC = 32                   # scan chunk length
H = 12                    # warmup steps (0.5^H carry error ~2.4e-4)
NH = 2                    # T halves
TH = T // NH              # 2048
NCHH = TH // LC           # 64 chunks per batch per half
PADX = 16                 # conv left halo (max (K-1)*d)
SX = PADX + T             # 4112
PADH = LC                 # XH head zeros (warmup reads cols [LC-H, LC))
SXH = PADH + T            # 4128
# conv dtype per layer: True = f32r (1 cy/row, ~12-bit inputs), False = fp32
F32R_LAYER = (False, True, True, True)

_cache = {}


def _build(f32r_layers=F32R_LAYER):
    import concourse.bass as bass
    import concourse.bacc as bacc
    import concourse.tile as tile
    import concourse.mybir as mybir

    dt = mybir.dt.float32
    dtr = mybir.dt.float32r
    Alu = mybir.AluOpType
    Act = mybir.ActivationFunctionType

    nc = bacc.Bacc("TRN2", target_bir_lowering=False, debug=False)
    x_d = nc.dram_tensor("x", [BL, C, T], dtr, kind="ExternalInput")
    wt_d = nc.dram_tensor("wt", [C, 4, K, C], dtr, kind="ExternalInput")
    b_d = nc.dram_tensor("bias", [C, 4], dt, kind="ExternalInput")
    z_d = nc.dram_tensor("zpad", [C, BL * PADX], dtr, kind="ExternalInput")
    o_d = nc.dram_tensor("out", [BL, C, T], dtr, kind="ExternalOutput")

    with tile.TileContext(nc) as tc:
        with (
            tc.tile_pool(name="big", bufs=1) as big,
            tc.tile_pool(name="small", bufs=1) as small,
            tc.tile_pool(name="psum", bufs=4, space="PSUM") as pp,
        ):
            X = big.tile([C, BL, SX], dtr, tag="X")
            XH = big.tile([C, BL, SXH], dt, tag="XH")
            ATS = [big.tile([C, BL, TH], dt, name=f"AT{h}", tag=f"AT{h}") for h in range(NH)]
            WT = small.tile([C, 4, K, C], dtr, tag="WT")
            BIAS = small.tile([C, 4], dt, tag="BIAS")
            # two independent sub-chains (A: chunks [0,NC2), B: [NC2,NCHH)) so
            # consecutive DVE ops never chain RAW back-to-back
            NC2 = NCHH // 2
            VA = small.tile([C, BL, NC2], dt, tag="VA")
            VB = small.tile([C, BL, NC2], dt, tag="VB")
            SCRA = small.tile([C, BL, NC2], dt, tag="SCRA")
            SCRB = small.tile([C, BL, NC2], dt, tag="SCRB")

            nc.sync.dma_start(WT[:], wt_d[:])
            nc.sync.dma_start(BIAS[:], b_d[:])
            for b in range(BL):
                nc.sync.dma_start(X[:, b, 0:PADX], z_d[:, b * PADX : (b + 1) * PADX])
            nc.vector.memset(XH[:, :, 0:PADH], 0.0)
            for b in range(BL):
                for h in range(NH):
                    nc.sync.dma_start(
                        X[:, b, PADX + h * TH : PADX + (h + 1) * TH],
                        x_d[b][:, h * TH : (h + 1) * TH],
                    )

            XH4 = XH[:].rearrange("p a (c l) -> p a c l", l=LC)   # c: 129
            AT4S = [A[:].rearrange("p a (c l) -> p a c l", l=LC) for A in ATS]
            Xf = X[:].bitcast(dt)  # exact-bits view for fp32 layers

            def conv(li, d, h, use_f32r):
                # Layer 1 emits the first half-T's tiles (all batches) first so
                # the scan's sub-chain A starts after 8 evacs; later layers use
                # b-major order, which Tile coalesces into fewer sems.
                order = [(tt, b) for b in range(BL) for tt in range(TH // 512)]
                for tt, b in order:
                    if True:
                        t0 = h * TH + tt * 512
                        ps = pp.tile([C, 512], dt, tag="ps")
                        for k in range(K):
                            sh = (K - 1 - k) * d
                            if use_f32r:
                                rhs = X[:, b, PADX + t0 - sh : PADX + t0 - sh + 512]
                                lhsT = WT[:, li, k, :]
                            else:
                                rhs = Xf[:, b, PADX + t0 - sh : PADX + t0 - sh + 512]
                                lhsT = WT[:, li, k, :].bitcast(dt)
                            nc.tensor.matmul(
                                ps[:], lhsT, rhs, start=(k == 0), stop=(k == K - 1)
                            )
                        nc.scalar.activation(
                            XH[:, b, PADH + t0 : PADH + t0 + 512], ps[:],
                            Act.Identity, bias=BIAS[:, li : li + 1], scale=1.0,
                        )

            def scan(h):
                c0 = h * NCHH
                AT4 = AT4S[h]
                subs = ((0, VA, SCRA), (NC2, VB, SCRB))
                for j in range(H + LC):
                    jj = (LC - H) + j
                    cs, l = jj // LC, jj % LC
                    cols, dsts = [], []
                    for coff, V, SCR in subs:
                        cols.append(XH4[:, :, c0 + coff + cs : c0 + coff + cs + NC2, l])
                        dsts.append(SCR[:] if j < H else AT4[:, :, coff : coff + NC2, j - H])
                    # A = 0.5*v + xh  (sub-chains interleaved to hide RAW latency)
                    for (coff, V, SCR), col, dst in zip(subs, cols, dsts):
                        if j == 0:
                            nc.vector.scalar_tensor_tensor(
                                dst, col, 0.0, col, op0=Alu.mult, op1=Alu.add)
                        else:
                            nc.vector.scalar_tensor_tensor(
                                dst, V[:], 0.5, col, op0=Alu.mult, op1=Alu.add)
                    # v' = (A < 1) * A
                    if j < H + LC - 1:
                        for (coff, V, SCR), dst in zip(subs, dsts):
                            nc.vector.scalar_tensor_tensor(
                                V[:], dst, float(VTH), dst, op0=Alu.is_lt, op1=Alu.mult)

            def spike_res(li, h):
                # Per-batch ops so conv(li+1)/out-DMA of batch b start as soon
                # as batch b's X is updated (batch order matches conv order).
                AT = ATS[h]
                final = li == len(DILATIONS) - 1 and h == NH - 1
                for b in range(BL):
                    xs = X[:, b, PADX + h * TH : PADX + (h + 1) * TH]
                    if final:
                        # final half on DVE (otherwise exposed at the tail)
                        nc.vector.scalar_tensor_tensor(
                            xs, AT[:, b, :], float(VTH), xs, op0=Alu.is_ge, op1=Alu.add)
                    else:
                        # Pool: s = (A >= 1) overwrites AT in place, then X += s
                        nc.gpsimd.tensor_scalar(
                            AT[:, b, :], AT[:, b, :], float(VTH), 1.0,
                            op0=Alu.is_ge, op1=Alu.mult)
                        nc.gpsimd.tensor_tensor(xs, AT[:, b, :], xs, op=Alu.add)

            # Issue order: both halves' convs precede the layer's sr ops so the
            # h=1 conv halo reads pre-spike X (correctness) and PE pipelines
            # ahead of the scans (conv(li,1) runs during scan(li,0), and
            # conv(li+1,0) during scan(li,1)).
            for li, d in enumerate(DILATIONS):
                conv(li, d, 0, f32r_layers[li])
                conv(li, d, 1, f32r_layers[li])
                for h in range(NH):
                    scan(h)
                    spike_res(li, h)
                    if li == len(DILATIONS) - 1:
                        for b in range(BL):
                            nc.sync.dma_start(
                                o_d[b][:, h * TH : (h + 1) * TH],
                                X[:, b, PADX + h * TH : PADX + (h + 1) * TH],
                            )

    nc.compile()
    return nc


def kernel(x, w, gamma, beta, mean, var, **_):
    from concourse.bass_utils import run_bass_kernel_spmd

    x = np.ascontiguousarray(x, np.float32)
    inv = (gamma / np.sqrt(var + EPS)).astype(np.float32)          # [4, C]
    # wt[ci, l, k, co] = 0.5 * w[l, co, ci, k] * inv[l, co]
    wt = (0.5 * w * inv[:, :, None, None]).astype(np.float32)      # [4, Co, Ci, K]
    wt = np.ascontiguousarray(wt.transpose(2, 0, 3, 1))            # [Ci, 4, K, Co]
    bias = (0.5 * (beta - mean * inv)).astype(np.float32).T        # [C, 4]
    bias = np.ascontiguousarray(bias)

    if "nc" not in _cache:
        _cache["nc"] = _build()
    nc = _cache["nc"]

    zpad = np.zeros((C, BL * PADX), np.float32)
    in_maps = [
        {"x": np.ascontiguousarray(x[i * BL : (i + 1) * BL]), "wt": wt, "bias": bias,
         "zpad": zpad}
        for i in range(NCORES)
    ]
    res = run_bass_kernel_spmd(nc, in_maps, list(range(NCORES)))
    return np.concatenate([res.results[i]["out"] for i in range(NCORES)], axis=0)


# revision 3
# speedup vs baseline: 1.0092x; 1.0092x over previous
"""Trainium2 Bass kernel v2 for LongRangeTCN.

Per core (BL=4 batches), per layer, per T-half (TH=2048):
  conv: 3-tap dilated conv as PSUM-accumulated matmuls (per-layer fp32 or f32r),
        Act engine evacuates PSUM->XH adding folded BN bias (xh = 0.5*BN(conv)).
  scan: LIF wavefront on DVE over chunks of LC=32 with H=12 warmup
        (2 scalar_tensor_tensor ops per step, all 4 batches = 256 cols wide).
        The A-trajectory goes to a separate AT tile so XH stays read-only.
  sr:   spike + residual X += (A >= 1) on Pool/GpSimd (TS extract + TT add; the
        95ns Q7 launch amortizes over full-width ops). X is float32r so
        f32r-layer matmuls accept it; fp32 layers read a bitcast-fp32 view.
Pipeline: per-engine program order makes conv(li,h=1) on PE run during
scan(li,h=0) on DVE, Pool's sr(li,h) during scan(li,h+1).
"""

import numpy as np

TAU, VTH, EPS, K = 2.0, 1.0, 1e-5, 3
DILATIONS = (1, 2, 4, 8)
B, C, T = 32, 128, 4096
NCORES = 8
BL = B // NCORES          # 4 batches per core
LC = 32                   # scan chunk length
H = 12                    # warmup steps (0.5^H carry error ~2.4e-4)
NH = 2                    # T halves
TH = T // NH              # 2048
NCHH = TH // LC           # 64 chunks per batch per half
PADX = 16                 # conv left halo (max (K-1)*d)
SX = PADX + T             # 4112
PADH = LC                 # XH head zeros (warmup reads cols [LC-H, LC))
SXH = PADH + T            # 4128
# conv dtype per layer: True = f32r (1 cy/row, ~12-bit inputs), False = fp32
F32R_LAYER = (False, False, False, False)

_cache = {}


def _build(f32r_layers=F32R_LAYER):
    import concourse.bass as bass
    import concourse.bacc as bacc
    import concourse.tile as tile
    import concourse.mybir as mybir

    dt = mybir.dt.float32
    dtr = mybir.dt.float32r
    Alu = mybir.AluOpType
    Act = mybir.ActivationFunctionType

    any_f32r = any(f32r_layers)
    dtx = dtr if any_f32r else dt  # X/weights dtype: f32r only when needed
    nc = bacc.Bacc("TRN2", target_bir_lowering=False, debug=False)
    x_d = nc.dram_tensor("x", [BL, C, T], dtx, kind="ExternalInput")
    wt_d = nc.dram_tensor("wt", [C, 4, K, C], dtx, kind="ExternalInput")
    b_d = nc.dram_tensor("bias", [C, 4], dt, kind="ExternalInput")
    if any_f32r:
        z_d = nc.dram_tensor("zpad", [C, BL * PADX], dtx, kind="ExternalInput")
    o_d = nc.dram_tensor("out", [BL, C, T], dtx, kind="ExternalOutput")

    with tile.TileContext(nc) as tc:
        with (
            tc.tile_pool(name="big", bufs=1) as big,
            tc.tile_pool(name="small", bufs=1) as small,
            tc.tile_pool(name="psum", bufs=4, space="PSUM") as pp,
        ):
            X = big.tile([C, BL, SX], dtx, tag="X")
            XH = big.tile([C, BL, SXH], dt, tag="XH")
            ATS = [big.tile([C, BL, TH], dt, name=f"AT{h}", tag=f"AT{h}") for h in range(NH)]
            WT = small.tile([C, 4, K, C], dtx, tag="WT")
            BIAS = small.tile([C, 4], dt, tag="BIAS")
            # two independent sub-chains (A: chunks [0,NC2), B: [NC2,NCHH)) so
            # consecutive DVE ops never chain RAW back-to-back
            NC2 = NCHH // 2
            VA = small.tile([C, BL, NC2], dt, tag="VA")
            VB = small.tile([C, BL, NC2], dt, tag="VB")
            SCRA = small.tile([C, BL, NC2], dt, tag="SCRA")
            SCRB = small.tile([C, BL, NC2], dt, tag="SCRB")

            nc.sync.dma_start(WT[:], wt_d[:])
            nc.sync.dma_start(BIAS[:], b_d[:])
            if any_f32r:
                for b in range(BL):
                    nc.sync.dma_start(X[:, b, 0:PADX], z_d[:, b * PADX : (b + 1) * PADX])
            else:
                nc.vector.memset(X[:, :, 0:PADX], 0.0)
            nc.vector.memset(XH[:, :, 0:PADH], 0.0)
            for b in range(BL):
                for h in range(NH):
                    nc.sync.dma_start(
                        X[:, b, PADX + h * TH : PADX + (h + 1) * TH],
                        x_d[b][:, h * TH : (h + 1) * TH],
                    )

            XH4 = XH[:].rearrange("p a (c l) -> p a c l", l=LC)   # c: 129
            AT4S = [A[:].rearrange("p a (c l) -> p a c l", l=LC) for A in ATS]
            Xf = X[:].bitcast(dt)  # exact-bits view for fp32 layers

            def conv(li, d, h, use_f32r):
                # Layer 1 emits the first half-T's tiles (all batches) first so
                # the scan's sub-chain A starts after 8 evacs; later layers use
                # b-major order, which Tile coalesces into fewer sems.
                order = [(tt, b) for b in range(BL) for tt in range(TH // 512)]
                for tt, b in order:
                    if True:
                        t0 = h * TH + tt * 512
                        ps = pp.tile([C, 512], dt, tag="ps")
                        for k in range(K):
                            sh = (K - 1 - k) * d
                            if use_f32r:
                                rhs = X[:, b, PADX + t0 - sh : PADX + t0 - sh + 512]
                                lhsT = WT[:, li, k, :]
                            else:
                                rhs = Xf[:, b, PADX + t0 - sh : PADX + t0 - sh + 512]
                                lhsT = WT[:, li, k, :].bitcast(dt)
                            nc.tensor.matmul(
                                ps[:], lhsT, rhs, start=(k == 0), stop=(k == K - 1)
                            )
                        nc.scalar.activation(
                            XH[:, b, PADH + t0 : PADH + t0 + 512], ps[:],
                            Act.Identity, bias=BIAS[:, li : li + 1], scale=1.0,
                        )

            def scan(h):
                c0 = h * NCHH
                AT4 = AT4S[h]
                subs = ((0, VA, SCRA), (NC2, VB, SCRB))
                for j in range(H + LC):
                    jj = (LC - H) + j
                    cs, l = jj // LC, jj % LC
                    cols, dsts = [], []
                    for coff, V, SCR in subs:
                        cols.append(XH4[:, :, c0 + coff + cs : c0 + coff + cs + NC2, l])
                        dsts.append(SCR[:] if j < H else AT4[:, :, coff : coff + NC2, j - H])
                    # A = 0.5*v + xh  (sub-chains interleaved to hide RAW latency)
                    for (coff, V, SCR), col, dst in zip(subs, cols, dsts):
                        if j == 0:
                            nc.vector.scalar_tensor_tensor(
                                dst, col, 0.0, col, op0=Alu.mult, op1=Alu.add)
                        else:
                            nc.vector.scalar_tensor_tensor(
                                dst, V[:], 0.5, col, op0=Alu.mult, op1=Alu.add)
                    # v' = (A < 1) * A
                    if j < H + LC - 1:
                        for (coff, V, SCR), dst in zip(subs, dsts):
                            nc.vector.scalar_tensor_tensor(
                                V[:], dst, float(VTH), dst, op0=Alu.is_lt, op1=Alu.mult)

            def spike_res(li, h):
                # Per-batch ops so conv(li+1)/out-DMA of batch b start as soon
                # as batch b's X is updated (batch order matches conv order).
                AT = ATS[h]
                final = li == len(DILATIONS) - 1 and h == NH - 1
                for b in range(BL):
                    xs = X[:, b, PADX + h * TH : PADX + (h + 1) * TH]
                    if final:
                        nc.vector.scalar_tensor_tensor(
                            xs, AT[:, b, :], float(VTH), xs, op0=Alu.is_ge, op1=Alu.add)
                    else:
                        # Pool: s = (A >= 1) overwrites AT in place, then X += s
                        nc.gpsimd.tensor_scalar(
                            AT[:, b, :], AT[:, b, :], float(VTH), 1.0,
                            op0=Alu.is_ge, op1=Alu.mult)
                        nc.gpsimd.tensor_tensor(xs, AT[:, b, :], xs, op=Alu.add)

            # Issue order: both halves' convs precede the layer's sr ops so the
            # h=1 conv halo reads pre-spike X (correctness) and PE pipelines
            # ahead of the scans (conv(li,1) runs during scan(li,0), and
            # conv(li+1,0) during scan(li,1)).
            for li, d in enumerate(DILATIONS):
                conv(li, d, 0, f32r_layers[li])
                conv(li, d, 1, f32r_layers[li])
                for h in range(NH):
                    scan(h)
                    spike_res(li, h)
                    if li == len(DILATIONS) - 1:
                        for b in range(BL):
                            nc.sync.dma_start(
                                o_d[b][:, h * TH : (h + 1) * TH],
                                X[:, b, PADX + h * TH : PADX + (h + 1) * TH],
                            )

    nc.compile()
    return nc


def kernel(x, w, gamma, beta, mean, var, **_):
    from concourse.bass_utils import run_bass_kernel_spmd

    x = np.ascontiguousarray(x, np.float32)
    inv = (gamma / np.sqrt(var + EPS)).astype(np.float32)          # [4, C]
    # wt[ci, l, k, co] = 0.5 * w[l, co, ci, k] * inv[l, co]
    wt = (0.5 * w * inv[:, :, None, None]).astype(np.float32)      # [4, Co, Ci, K]
    wt = np.ascontiguousarray(wt.transpose(2, 0, 3, 1))            # [Ci, 4, K, Co]
    bias = (0.5 * (beta - mean * inv)).astype(np.float32).T        # [C, 4]
    bias = np.ascontiguousarray(bias)

    if "nc" not in _cache:
        _cache["nc"] = _build()
    nc = _cache["nc"]

    extra = {}
    if any(F32R_LAYER):
        extra["zpad"] = np.zeros((C, BL * PADX), np.float32)
    in_maps = [
        {"x": np.ascontiguousarray(x[i * BL : (i + 1) * BL]), "wt": wt, "bias": bias,
         **extra}
        for i in range(NCORES)
    ]
    res = run_bass_kernel_spmd(nc, in_maps, list(range(NCORES)))
    return np.concatenate([res.results[i]["out"] for i in range(NCORES)], axis=0)


# revision 4
# speedup vs baseline: 1.0157x; 1.0064x over previous
"""Trainium2 Bass kernel v2 for LongRangeTCN.

Per core (BL=4 batches), per layer, per T-half (TH=2048):
  conv: 3-tap dilated conv as PSUM-accumulated matmuls (per-layer fp32 or f32r),
        Act engine evacuates PSUM->XH adding folded BN bias (xh = 0.5*BN(conv)).
  scan: LIF wavefront on DVE over chunks of LC=32 with H=12 warmup
        (2 scalar_tensor_tensor ops per step, all 4 batches = 256 cols wide).
        The A-trajectory goes to a separate AT tile so XH stays read-only.
  sr:   spike + residual X += (A >= 1) on Pool/GpSimd (TS extract + TT add; the
        95ns Q7 launch amortizes over full-width ops). X is float32r so
        f32r-layer matmuls accept it; fp32 layers read a bitcast-fp32 view.
Pipeline: per-engine program order makes conv(li,h=1) on PE run during
scan(li,h=0) on DVE, Pool's sr(li,h) during scan(li,h+1).
"""

import numpy as np

TAU, VTH, EPS, K = 2.0, 1.0, 1e-5, 3
DILATIONS = (1, 2, 4, 8)
B, C, T = 32, 128, 4096
NCORES = 8
BL = B // NCORES          # 4 batches per core
LC = 32                   # scan chunk length
H = 12                    # warmup steps (0.5^H carry error ~2.4e-4)
NH = 2                    # T halves
TH = T // NH              # 2048
NCHH = TH // LC           # 64 chunks per batch per half
PADX = 16                 # conv left halo (max (K-1)*d)
SX = PADX + T             # 4112
PADH = LC                 # XH head zeros (warmup reads cols [LC-H, LC))
SXH = PADH + T            # 4128
# conv dtype per layer: True = f32r (1 cy/row, ~12-bit inputs), False = fp32
F32R_LAYER = (False, False, False, False)

_cache = {}


def _build(f32r_layers=F32R_LAYER):
    import concourse.bass as bass
    import concourse.bacc as bacc
    import concourse.tile as tile
    import concourse.mybir as mybir

    dt = mybir.dt.float32
    dtr = mybir.dt.float32r
    Alu = mybir.AluOpType
    Act = mybir.ActivationFunctionType

    any_f32r = any(f32r_layers)
    dtx = dtr if any_f32r else dt  # X/weights dtype: f32r only when needed
    nc = bacc.Bacc("TRN2", target_bir_lowering=False, debug=False)
    x_d = nc.dram_tensor("x", [BL, C, T], dtx, kind="ExternalInput")
    wt_d = nc.dram_tensor("wt", [C, 4, K, C], dtx, kind="ExternalInput")
    b_d = nc.dram_tensor("bias", [C, 4], dt, kind="ExternalInput")
    if any_f32r:
        z_d = nc.dram_tensor("zpad", [C, BL * PADX], dtx, kind="ExternalInput")
    o_d = nc.dram_tensor("out", [BL, C, T], dtx, kind="ExternalOutput")

    with tile.TileContext(nc) as tc:
        with (
            tc.tile_pool(name="big", bufs=1) as big,
            tc.tile_pool(name="small", bufs=1) as small,
            tc.tile_pool(name="psum", bufs=4, space="PSUM") as pp,
        ):
            X = big.tile([C, BL, SX], dtx, tag="X")
            XH = big.tile([C, BL, SXH], dt, tag="XH")
            ATS = [big.tile([C, BL, TH], dt, name=f"AT{h}", tag=f"AT{h}") for h in range(NH)]
            WT = small.tile([C, 4, K, C], dtx, tag="WT")
            BIAS = small.tile([C, 4], dt, tag="BIAS")
            # two independent sub-chains (A: chunks [0,NC2), B: [NC2,NCHH)) so
            # consecutive DVE ops never chain RAW back-to-back
            NC2 = NCHH // 2
            VA = small.tile([C, BL, NC2], dt, tag="VA")
            VB = small.tile([C, BL, NC2], dt, tag="VB")
            SCRA = small.tile([C, BL, NC2], dt, tag="SCRA")
            SCRB = small.tile([C, BL, NC2], dt, tag="SCRB")
            # Pool tail-scan slice state (final layer h=1): vv = v/2 chain
            CP2 = 40  # chunks/batch handled by Pool in the final scan
            VP = small.tile([C, BL, CP2], dt, tag="VP")
            MP = small.tile([C, BL, CP2], dt, tag="MP")
            SCRP = small.tile([C, BL, CP2], dt, tag="SCRP")

            # layer-1 weights first so the first conv isn't queued behind the
            # full weight load
            nc.sync.dma_start(WT[:, 0], wt_d[:, 0])
            nc.sync.dma_start(BIAS[:], b_d[:])
            if any_f32r:
                for b in range(BL):
                    nc.sync.dma_start(X[:, b, 0:PADX], z_d[:, b * PADX : (b + 1) * PADX])
            else:
                nc.vector.memset(X[:, :, 0:PADX], 0.0)
            nc.vector.memset(XH[:, :, 0:PADH], 0.0)
            for b in range(BL):
                for h in range(NH):
                    nc.sync.dma_start(
                        X[:, b, PADX + h * TH : PADX + (h + 1) * TH],
                        x_d[b][:, h * TH : (h + 1) * TH],
                    )
            nc.sync.dma_start(WT[:, 1:4], wt_d[:, 1:4])

            XH4 = XH[:].rearrange("p a (c l) -> p a c l", l=LC)   # c: 129
            AT4S = [A[:].rearrange("p a (c l) -> p a c l", l=LC) for A in ATS]
            Xf = X[:].bitcast(dt)  # exact-bits view for fp32 layers

            def conv(li, d, h, use_f32r, thalf_major=False):
                # b-major order coalesces best in steady state; the final
                # conv uses half-T-major so the last scan's first half-
                # wavefront starts ~20us before the last evacs land.
                if thalf_major:
                    order = [(th2 * 2 + tt, b) for th2 in range(2)
                             for b in range(BL) for tt in range(2)]
                else:
                    order = [(tt, b) for b in range(BL) for tt in range(TH // 512)]
                for tt, b in order:
                    if True:
                        t0 = h * TH + tt * 512
                        ps = pp.tile([C, 512], dt, tag="ps")
                        for k in range(K):
                            sh = (K - 1 - k) * d
                            if use_f32r:
                                rhs = X[:, b, PADX + t0 - sh : PADX + t0 - sh + 512]
                                lhsT = WT[:, li, k, :]
                            else:
                                rhs = Xf[:, b, PADX + t0 - sh : PADX + t0 - sh + 512]
                                lhsT = WT[:, li, k, :].bitcast(dt)
                            nc.tensor.matmul(
                                ps[:], lhsT, rhs, start=(k == 0), stop=(k == K - 1)
                            )
                        nc.scalar.activation(
                            XH[:, b, PADH + t0 : PADH + t0 + 512], ps[:],
                            Act.Identity, bias=BIAS[:, li : li + 1], scale=1.0,
                        )

            def pool_slice(h, base):
                # Pool vv=v/2 chain over chunks [base, NCHH) of every batch in
                # half h; starts while DVE is still on the previous scan.
                c0 = h * NCHH + base
                AT4 = AT4S[h]
                W = NCHH - base
                for j in range(H + LC):
                    jj = (LC - H) + j
                    cs, l = jj // LC, jj % LC
                    col = XH4[:, :, c0 + cs : c0 + cs + W, l]
                    dst = SCRP[:, :, 0:W] if j < H else AT4[:, :, base:NCHH, j - H]
                    if j == 0:
                        nc.gpsimd.tensor_scalar(
                            dst, col, 1.0, 0.0, op0=Alu.mult, op1=Alu.add)
                    else:
                        nc.gpsimd.tensor_tensor(dst, VP[:, :, 0:W], col, op=Alu.add)
                    if j < H + LC - 1:
                        nc.gpsimd.tensor_scalar(
                            MP[:, :, 0:W], dst, float(VTH), 0.5,
                            op0=Alu.is_lt, op1=Alu.mult)
                        nc.gpsimd.tensor_tensor(
                            VP[:, :, 0:W], MP[:, :, 0:W], dst, op=Alu.mult)

            def scan(h, nchunks=NCHH, cbase=0):
                c0 = h * NCHH + cbase
                AT4 = AT4S[h]
                nc2 = nchunks // 2
                subs = ((0, VA, SCRA), (nc2, VB, SCRB))
                NC2l = nc2
                for j in range(H + LC):
                    jj = (LC - H) + j
                    cs, l = jj // LC, jj % LC
                    cols, dsts = [], []
                    for coff, V, SCR in subs:
                        cols.append(XH4[:, :, c0 + coff + cs : c0 + coff + cs + NC2l, l])
                        dsts.append(SCR[:, :, 0:NC2l] if j < H
                                    else AT4[:, :, cbase + coff : cbase + coff + NC2l, j - H])
                    # A = 0.5*v + xh  (sub-chains interleaved to hide RAW latency)
                    for (coff, V, SCR), col, dst in zip(subs, cols, dsts):
                        if j == 0:
                            nc.vector.scalar_tensor_tensor(
                                dst, col, 0.0, col, op0=Alu.mult, op1=Alu.add)
                        else:
                            nc.vector.scalar_tensor_tensor(
                                dst, V[:, :, 0:NC2l], 0.5, col, op0=Alu.mult, op1=Alu.add)
                    # v' = (A < 1) * A
                    if j < H + LC - 1:
                        for (coff, V, SCR), dst in zip(subs, dsts):
                            nc.vector.scalar_tensor_tensor(
                                V[:, :, 0:NC2l], dst, float(VTH), dst,
                                op0=Alu.is_lt, op1=Alu.mult)

            def spike_res(li, h):
                # Per-batch ops so conv(li+1)/out-DMA of batch b start as soon
                # as batch b's X is updated (batch order matches conv order).
                AT = ATS[h]
                final = li == len(DILATIONS) - 1 and h == NH - 1
                for b in range(BL):
                    xs = X[:, b, PADX + h * TH : PADX + (h + 1) * TH]
                    if final:
                        # half-batch granularity: each output DMA starts ~1us
                        # after its half's spikes land
                        T2 = TH // 2
                        for g in range(2):
                            xg = X[:, b, PADX + h * TH + g * T2 : PADX + h * TH + (g + 1) * T2]
                            nc.vector.scalar_tensor_tensor(
                                xg, AT[:, b, g * T2 : (g + 1) * T2], float(VTH), xg,
                                op0=Alu.is_ge, op1=Alu.add)
                    else:
                        # Pool: s = (A >= 1) overwrites AT in place, then X += s
                        nc.gpsimd.tensor_scalar(
                            AT[:, b, :], AT[:, b, :], float(VTH), 1.0,
                            op0=Alu.is_ge, op1=Alu.mult)
                        nc.gpsimd.tensor_tensor(xs, AT[:, b, :], xs, op=Alu.add)

            # Issue order: both halves' convs precede the layer's sr ops so the
            # h=1 conv halo reads pre-spike X (correctness) and PE pipelines
            # ahead of the scans (conv(li,1) runs during scan(li,0), and
            # conv(li+1,0) during scan(li,1)).
            LAST = len(DILATIONS) - 1
            for li, d in enumerate(DILATIONS):
                conv(li, d, 0, f32r_layers[li])
                conv(li, d, 1, f32r_layers[li])
                for h in range(NH):
                    scan(h)
                    spike_res(li, h)
                    if li == LAST:
                        for b in range(BL):
                            if h == NH - 1:
                                # final half: half-batch DMAs on alternating
                                # queues, following the finer sr ops
                                T2 = TH // 2
                                for g in range(2):
                                    q = (nc.sync, nc.scalar)[(2 * b + g) % 2]
                                    q.dma_start(
                                        o_d[b][:, h * TH + g * T2 : h * TH + (g + 1) * T2],
                                        X[:, b, PADX + h * TH + g * T2 : PADX + h * TH + (g + 1) * T2],
                                    )
                            else:
                                nc.sync.dma_start(
                                    o_d[b][:, h * TH : (h + 1) * TH],
                                    X[:, b, PADX + h * TH : PADX + (h + 1) * TH],
                                )

    nc.compile()
    return nc


def kernel(x, w, gamma, beta, mean, var, **_):
    from concourse.bass_utils import run_bass_kernel_spmd

    x = np.ascontiguousarray(x, np.float32)
    inv = (gamma / np.sqrt(var + EPS)).astype(np.float32)          # [4, C]
    # wt[ci, l, k, co] = 0.5 * w[l, co, ci, k] * inv[l, co]
    wt = (0.5 * w * inv[:, :, None, None]).astype(np.float32)      # [4, Co, Ci, K]
    wt = np.ascontiguousarray(wt.transpose(2, 0, 3, 1))            # [Ci, 4, K, Co]
    bias = (0.5 * (beta - mean * inv)).astype(np.float32).T        # [C, 4]
    bias = np.ascontiguousarray(bias)

    if "nc" not in _cache:
        _cache["nc"] = _build()
    nc = _cache["nc"]

    extra = {}
    if any(F32R_LAYER):
        extra["zpad"] = np.zeros((C, BL * PADX), np.float32)
    in_maps = [
        {"x": np.ascontiguousarray(x[i * BL : (i + 1) * BL]), "wt": wt, "bias": bias,
         **extra}
        for i in range(NCORES)
    ]
    res = run_bass_kernel_spmd(nc, in_maps, list(range(NCORES)))
    return np.concatenate([res.results[i]["out"] for i in range(NCORES)], axis=0)


# revision 5
# speedup vs baseline: 1.0319x; 1.0160x over previous
"""Trainium2 Bass kernel v2 for LongRangeTCN.

Per core (BL=4 batches), per layer, per T-half (TH=2048):
  conv: 3-tap dilated conv as PSUM-accumulated matmuls (per-layer fp32 or f32r),
        Act engine evacuates PSUM->XH adding folded BN bias (xh = 0.5*BN(conv)).
  scan: LIF wavefront on DVE over chunks of LC=32 with H=12 warmup
        (2 scalar_tensor_tensor ops per step, all 4 batches = 256 cols wide).
        The A-trajectory goes to a separate AT tile so XH stays read-only.
  sr:   spike + residual X += (A >= 1) on Pool/GpSimd (TS extract + TT add; the
        95ns Q7 launch amortizes over full-width ops). X is float32r so
        f32r-layer matmuls accept it; fp32 layers read a bitcast-fp32 view.
Pipeline: per-engine program order makes conv(li,h=1) on PE run during
scan(li,h=0) on DVE, Pool's sr(li,h) during scan(li,h+1).
"""

import numpy as np

TAU, VTH, EPS, K = 2.0, 1.0, 1e-5, 3
DILATIONS = (1, 2, 4, 8)
B, C, T = 32, 128, 4096
NCORES = 8
BL = B // NCORES          # 4 batches per core
LC = 32                   # scan chunk length
H = 12                    # warmup steps (0.5^H carry error ~2.4e-4)
NH = 2                    # T halves
TH = T // NH              # 2048
NCHH = TH // LC           # 64 chunks per batch per half
PADX = 16                 # conv left halo (max (K-1)*d)
SX = PADX + T             # 4112
PADH = LC                 # XH head zeros (warmup reads cols [LC-H, LC))
SXH = PADH + T            # 4128
# conv dtype per layer: True = f32r (1 cy/row, ~12-bit inputs), False = fp32
F32R_LAYER = (False, False, False, False)

_cache = {}


def _build(f32r_layers=F32R_LAYER):
    import concourse.bass as bass
    import concourse.bacc as bacc
    import concourse.tile as tile
    import concourse.mybir as mybir

    dt = mybir.dt.float32
    dtr = mybir.dt.float32r
    Alu = mybir.AluOpType
    Act = mybir.ActivationFunctionType

    any_f32r = any(f32r_layers)
    dtx = dtr if any_f32r else dt  # X/weights dtype: f32r only when needed
    nc = bacc.Bacc("TRN2", target_bir_lowering=False, debug=False)
    x_d = nc.dram_tensor("x", [BL, C, T], dtx, kind="ExternalInput")
    wt_d = nc.dram_tensor("wt", [C, 4, K, C], dtx, kind="ExternalInput")
    b_d = nc.dram_tensor("bias", [C, 4], dt, kind="ExternalInput")
    if any_f32r:
        z_d = nc.dram_tensor("zpad", [C, BL * PADX], dtx, kind="ExternalInput")
    o_d = nc.dram_tensor("out", [BL, C, T], dtx, kind="ExternalOutput")

    with tile.TileContext(nc) as tc:
        with (
            tc.tile_pool(name="big", bufs=1) as big,
            tc.tile_pool(name="small", bufs=1) as small,
            tc.tile_pool(name="psum", bufs=4, space="PSUM") as pp,
        ):
            X = big.tile([C, BL, SX], dtx, tag="X")
            XH = big.tile([C, BL, SXH], dt, tag="XH")
            ATS = [big.tile([C, BL, TH], dt, name=f"AT{h}", tag=f"AT{h}") for h in range(NH)]
            WT = small.tile([C, 4, K, C], dtx, tag="WT")
            BIAS = small.tile([C, 4], dt, tag="BIAS")
            # two independent sub-chains (A: chunks [0,NC2), B: [NC2,NCHH)) so
            # consecutive DVE ops never chain RAW back-to-back
            NC2 = NCHH // 2
            VA = small.tile([C, BL, NC2], dt, tag="VA")
            VB = small.tile([C, BL, NC2], dt, tag="VB")
            SCRA = small.tile([C, BL, NC2], dt, tag="SCRA")
            SCRB = small.tile([C, BL, NC2], dt, tag="SCRB")
            # Pool tail-scan slice state (final layer h=1): vv = v/2 chain
            CP2 = 40  # chunks/batch handled by Pool in the final scan
            VP = small.tile([C, BL, CP2], dt, tag="VP")
            MP = small.tile([C, BL, CP2], dt, tag="MP")
            SCRP = small.tile([C, BL, CP2], dt, tag="SCRP")

            # layer-1 weights first so the first conv isn't queued behind the
            # full weight load
            nc.sync.dma_start(WT[:, 0], wt_d[:, 0])
            nc.sync.dma_start(BIAS[:], b_d[:])
            if any_f32r:
                for b in range(BL):
                    nc.sync.dma_start(X[:, b, 0:PADX], z_d[:, b * PADX : (b + 1) * PADX])
            else:
                nc.vector.memset(X[:, :, 0:PADX], 0.0)
            nc.vector.memset(XH[:, :, 0:PADH], 0.0)
            for b in range(BL):
                for h in range(NH):
                    nc.sync.dma_start(
                        X[:, b, PADX + h * TH : PADX + (h + 1) * TH],
                        x_d[b][:, h * TH : (h + 1) * TH],
                    )
            nc.sync.dma_start(WT[:, 1:4], wt_d[:, 1:4])

            XH4 = XH[:].rearrange("p a (c l) -> p a c l", l=LC)   # c: 129
            AT4S = [A[:].rearrange("p a (c l) -> p a c l", l=LC) for A in ATS]
            Xf = X[:].bitcast(dt)  # exact-bits view for fp32 layers

            def conv(li, d, h, use_f32r, thalf_major=False):
                # b-major order coalesces best in steady state; the final
                # conv uses half-T-major so the last scan's first half-
                # wavefront starts ~20us before the last evacs land.
                if thalf_major:
                    order = [(th2 * 2 + tt, b) for th2 in range(2)
                             for b in range(BL) for tt in range(2)]
                else:
                    order = [(tt, b) for b in range(BL) for tt in range(TH // 512)]
                for tt, b in order:
                    if True:
                        t0 = h * TH + tt * 512
                        ps = pp.tile([C, 512], dt, tag="ps")
                        for k in range(K):
                            sh = (K - 1 - k) * d
                            if use_f32r:
                                rhs = X[:, b, PADX + t0 - sh : PADX + t0 - sh + 512]
                                lhsT = WT[:, li, k, :]
                            else:
                                rhs = Xf[:, b, PADX + t0 - sh : PADX + t0 - sh + 512]
                                lhsT = WT[:, li, k, :].bitcast(dt)
                            nc.tensor.matmul(
                                ps[:], lhsT, rhs, start=(k == 0), stop=(k == K - 1)
                            )
                        nc.scalar.activation(
                            XH[:, b, PADH + t0 : PADH + t0 + 512], ps[:],
                            Act.Identity, bias=BIAS[:, li : li + 1], scale=1.0,
                        )

            def pool_slice(h, base):
                # Pool vv=v/2 chain over chunks [base, NCHH) of every batch in
                # half h; starts while DVE is still on the previous scan.
                c0 = h * NCHH + base
                AT4 = AT4S[h]
                W = NCHH - base
                for j in range(H + LC):
                    jj = (LC - H) + j
                    cs, l = jj // LC, jj % LC
                    col = XH4[:, :, c0 + cs : c0 + cs + W, l]
                    dst = SCRP[:, :, 0:W] if j < H else AT4[:, :, base:NCHH, j - H]
                    if j == 0:
                        nc.gpsimd.tensor_scalar(
                            dst, col, 1.0, 0.0, op0=Alu.mult, op1=Alu.add)
                    else:
                        nc.gpsimd.tensor_tensor(dst, VP[:, :, 0:W], col, op=Alu.add)
                    if j < H + LC - 1:
                        nc.gpsimd.tensor_scalar(
                            MP[:, :, 0:W], dst, float(VTH), 0.5,
                            op0=Alu.is_lt, op1=Alu.mult)
                        nc.gpsimd.tensor_tensor(
                            VP[:, :, 0:W], MP[:, :, 0:W], dst, op=Alu.mult)

            def scan(h, nchunks=NCHH, cbase=0, warm=H):
                c0 = h * NCHH + cbase
                AT4 = AT4S[h]
                nc2 = nchunks // 2
                subs = ((0, VA, SCRA), (nc2, VB, SCRB))
                NC2l = nc2
                for j in range(warm + LC):
                    jj = (LC - warm) + j
                    cs, l = jj // LC, jj % LC
                    cols, dsts = [], []
                    for coff, V, SCR in subs:
                        cols.append(XH4[:, :, c0 + coff + cs : c0 + coff + cs + NC2l, l])
                        dsts.append(SCR[:, :, 0:NC2l] if j < warm
                                    else AT4[:, :, cbase + coff : cbase + coff + NC2l, j - warm])
                    # A = 0.5*v + xh  (sub-chains interleaved to hide RAW latency)
                    for (coff, V, SCR), col, dst in zip(subs, cols, dsts):
                        if j == 0:
                            nc.vector.scalar_tensor_tensor(
                                dst, col, 0.0, col, op0=Alu.mult, op1=Alu.add)
                        else:
                            nc.vector.scalar_tensor_tensor(
                                dst, V[:, :, 0:NC2l], 0.5, col, op0=Alu.mult, op1=Alu.add)
                    # v' = (A < 1) * A
                    if j < warm + LC - 1:
                        for (coff, V, SCR), dst in zip(subs, dsts):
                            nc.vector.scalar_tensor_tensor(
                                V[:, :, 0:NC2l], dst, float(VTH), dst,
                                op0=Alu.is_lt, op1=Alu.mult)

            def spike_res(li, h):
                # Per-batch ops so conv(li+1)/out-DMA of batch b start as soon
                # as batch b's X is updated (batch order matches conv order).
                AT = ATS[h]
                final = li == len(DILATIONS) - 1 and h == NH - 1
                for b in range(BL):
                    xs = X[:, b, PADX + h * TH : PADX + (h + 1) * TH]
                    if final:
                        # half-batch granularity: each output DMA starts ~1us
                        # after its half's spikes land; last batch on Pool
                        # (idle by then) to shorten the serial DVE chain
                        T2 = TH // 2
                        for g in range(2):
                            xg = X[:, b, PADX + h * TH + g * T2 : PADX + h * TH + (g + 1) * T2]
                            ag = AT[:, b, g * T2 : (g + 1) * T2]
                            if b == BL - 1:
                                nc.gpsimd.tensor_scalar(
                                    ag, ag, float(VTH), 1.0, op0=Alu.is_ge, op1=Alu.mult)
                                nc.gpsimd.tensor_tensor(xg, ag, xg, op=Alu.add)
                            else:
                                nc.vector.scalar_tensor_tensor(
                                    xg, ag, float(VTH), xg, op0=Alu.is_ge, op1=Alu.add)
                    else:
                        # Pool: s = (A >= 1) overwrites AT in place, then X += s
                        nc.gpsimd.tensor_scalar(
                            AT[:, b, :], AT[:, b, :], float(VTH), 1.0,
                            op0=Alu.is_ge, op1=Alu.mult)
                        nc.gpsimd.tensor_tensor(xs, AT[:, b, :], xs, op=Alu.add)

            # Issue order: both halves' convs precede the layer's sr ops so the
            # h=1 conv halo reads pre-spike X (correctness) and PE pipelines
            # ahead of the scans (conv(li,1) runs during scan(li,0), and
            # conv(li+1,0) during scan(li,1)).
            LAST = len(DILATIONS) - 1
            for li, d in enumerate(DILATIONS):
                conv(li, d, 0, f32r_layers[li])
                conv(li, d, 1, f32r_layers[li])
                for h in range(NH):
                    # layer 4's warmup error cannot cascade (no layers after)
                    scan(h, warm=10 if li == LAST else H)
                    spike_res(li, h)
                    if li == LAST:
                        for b in range(BL):
                            if h == NH - 1:
                                # final half: half-batch DMAs on alternating
                                # queues, following the finer sr ops
                                T2 = TH // 2
                                for g in range(2):
                                    q = (nc.sync, nc.scalar)[(2 * b + g) % 2]
                                    q.dma_start(
                                        o_d[b][:, h * TH + g * T2 : h * TH + (g + 1) * T2],
                                        X[:, b, PADX + h * TH + g * T2 : PADX + h * TH + (g + 1) * T2],
                                    )
                            else:
                                nc.sync.dma_start(
                                    o_d[b][:, h * TH : (h + 1) * TH],
                                    X[:, b, PADX + h * TH : PADX + (h + 1) * TH],
                                )

    nc.compile()
    return nc


def kernel(x, w, gamma, beta, mean, var, **_):
    from concourse.bass_utils import run_bass_kernel_spmd

    x = np.ascontiguousarray(x, np.float32)
    inv = (gamma / np.sqrt(var + EPS)).astype(np.float32)          # [4, C]
    # wt[ci, l, k, co] = 0.5 * w[l, co, ci, k] * inv[l, co]
    wt = (0.5 * w * inv[:, :, None, None]).astype(np.float32)      # [4, Co, Ci, K]
    wt = np.ascontiguousarray(wt.transpose(2, 0, 3, 1))            # [Ci, 4, K, Co]
    bias = (0.5 * (beta - mean * inv)).astype(np.float32).T        # [C, 4]
    bias = np.ascontiguousarray(bias)

    if "nc" not in _cache:
        _cache["nc"] = _build()
    nc = _cache["nc"]

    extra = {}
    if any(F32R_LAYER):
        extra["zpad"] = np.zeros((C, BL * PADX), np.float32)
    in_maps = [
        {"x": np.ascontiguousarray(x[i * BL : (i + 1) * BL]), "wt": wt, "bias": bias,
         **extra}
        for i in range(NCORES)
    ]
    res = run_bass_kernel_spmd(nc, in_maps, list(range(NCORES)))
    return np.concatenate([res.results[i]["out"] for i in range(NCORES)], axis=0)


# revision 6
# speedup vs baseline: 1.0376x; 1.0055x over previous
"""Trainium2 Bass kernel v2 for LongRangeTCN.

Per core (BL=4 batches), per layer, per T-half (TH=2048):
  conv: 3-tap dilated conv as PSUM-accumulated matmuls (per-layer fp32 or f32r),
        Act engine evacuates PSUM->XH adding folded BN bias (xh = 0.5*BN(conv)).
  scan: LIF wavefront on DVE over chunks of LC=32 with H=12 warmup
        (2 scalar_tensor_tensor ops per step, all 4 batches = 256 cols wide).
        The A-trajectory goes to a separate AT tile so XH stays read-only.
  sr:   spike + residual X += (A >= 1) on Pool/GpSimd (TS extract + TT add; the
        95ns Q7 launch amortizes over full-width ops). X is float32r so
        f32r-layer matmuls accept it; fp32 layers read a bitcast-fp32 view.
Pipeline: per-engine program order makes conv(li,h=1) on PE run during
scan(li,h=0) on DVE, Pool's sr(li,h) during scan(li,h+1).
"""

import numpy as np

TAU, VTH, EPS, K = 2.0, 1.0, 1e-5, 3
DILATIONS = (1, 2, 4, 8)
B, C, T = 32, 128, 4096
NCORES = 8
BL = B // NCORES          # 4 batches per core
LC = 32                   # scan chunk length
H = 12                    # warmup steps (0.5^H carry error ~2.4e-4)
NH = 2                    # T halves
TH = T // NH              # 2048
NCHH = TH // LC           # 64 chunks per batch per half
PADX = 16                 # conv left halo (max (K-1)*d)
SX = PADX + T             # 4112
PADH = LC                 # XH head zeros (warmup reads cols [LC-H, LC))
SXH = PADH + T            # 4128
# conv dtype per layer: True = f32r (1 cy/row, ~12-bit inputs), False = fp32
F32R_LAYER = (False, False, False, False)

_cache = {}


def _build(f32r_layers=F32R_LAYER):
    import concourse.bass as bass
    import concourse.bacc as bacc
    import concourse.tile as tile
    import concourse.mybir as mybir

    dt = mybir.dt.float32
    dtr = mybir.dt.float32r
    Alu = mybir.AluOpType
    Act = mybir.ActivationFunctionType

    any_f32r = any(f32r_layers)
    dtx = dtr if any_f32r else dt  # X/weights dtype: f32r only when needed
    nc = bacc.Bacc("TRN2", target_bir_lowering=False, debug=False)
    x_d = nc.dram_tensor("x", [BL, C, T], dtx, kind="ExternalInput")
    wt_d = nc.dram_tensor("wt", [C, 4, K, C], dtx, kind="ExternalInput")
    b_d = nc.dram_tensor("bias", [C, 4], dt, kind="ExternalInput")
    if any_f32r:
        z_d = nc.dram_tensor("zpad", [C, BL * PADX], dtx, kind="ExternalInput")
    o_d = nc.dram_tensor("out", [BL, C, T], dtx, kind="ExternalOutput")

    with tile.TileContext(nc) as tc:
        with (
            tc.tile_pool(name="big", bufs=1) as big,
            tc.tile_pool(name="small", bufs=1) as small,
            tc.tile_pool(name="psum", bufs=4, space="PSUM") as pp,
        ):
            X = big.tile([C, BL, SX], dtx, tag="X")
            XH = big.tile([C, BL, SXH], dt, tag="XH")
            ATS = [big.tile([C, BL, TH], dt, name=f"AT{h}", tag=f"AT{h}") for h in range(NH)]
            WT = small.tile([C, 4, K, C], dtx, tag="WT")
            BIAS = small.tile([C, 4], dt, tag="BIAS")
            # two independent sub-chains (A: chunks [0,NC2), B: [NC2,NCHH)) so
            # consecutive DVE ops never chain RAW back-to-back
            NC2 = NCHH // 2
            VA = small.tile([C, BL, NC2], dt, tag="VA")
            VB = small.tile([C, BL, NC2], dt, tag="VB")
            SCRA = small.tile([C, BL, NC2], dt, tag="SCRA")
            SCRB = small.tile([C, BL, NC2], dt, tag="SCRB")
            # Pool tail-scan slice state (final layer h=1): vv = v/2 chain
            CP2 = 40  # chunks/batch handled by Pool in the final scan
            VP = small.tile([C, BL, CP2], dt, tag="VP")
            MP = small.tile([C, BL, CP2], dt, tag="MP")
            SCRP = small.tile([C, BL, CP2], dt, tag="SCRP")

            # layer-1 weights first so the first conv isn't queued behind the
            # full weight load
            nc.sync.dma_start(WT[:, 0], wt_d[:, 0])
            nc.sync.dma_start(BIAS[:], b_d[:])
            if any_f32r:
                for b in range(BL):
                    nc.sync.dma_start(X[:, b, 0:PADX], z_d[:, b * PADX : (b + 1) * PADX])
            else:
                nc.vector.memset(X[:, :, 0:PADX], 0.0)
            nc.vector.memset(XH[:, :, 0:PADH], 0.0)
            for b in range(BL):
                for h in range(NH):
                    nc.sync.dma_start(
                        X[:, b, PADX + h * TH : PADX + (h + 1) * TH],
                        x_d[b][:, h * TH : (h + 1) * TH],
                    )
            nc.sync.dma_start(WT[:, 1:4], wt_d[:, 1:4])

            XH4 = XH[:].rearrange("p a (c l) -> p a c l", l=LC)   # c: 129
            AT4S = [A[:].rearrange("p a (c l) -> p a c l", l=LC) for A in ATS]
            Xf = X[:].bitcast(dt)  # exact-bits view for fp32 layers

            def conv(li, d, h, use_f32r, thalf_major=False):
                # b-major order coalesces best in steady state; the final
                # conv uses half-T-major so the last scan's first half-
                # wavefront starts ~20us before the last evacs land.
                if thalf_major:
                    order = [(th2 * 2 + tt, b) for th2 in range(2)
                             for b in range(BL) for tt in range(2)]
                else:
                    order = [(tt, b) for b in range(BL) for tt in range(TH // 512)]
                for tt, b in order:
                    if True:
                        t0 = h * TH + tt * 512
                        ps = pp.tile([C, 512], dt, tag="ps")
                        for k in range(K):
                            sh = (K - 1 - k) * d
                            if use_f32r:
                                rhs = X[:, b, PADX + t0 - sh : PADX + t0 - sh + 512]
                                lhsT = WT[:, li, k, :]
                            else:
                                rhs = Xf[:, b, PADX + t0 - sh : PADX + t0 - sh + 512]
                                lhsT = WT[:, li, k, :].bitcast(dt)
                            nc.tensor.matmul(
                                ps[:], lhsT, rhs, start=(k == 0), stop=(k == K - 1)
                            )
                        nc.scalar.activation(
                            XH[:, b, PADH + t0 : PADH + t0 + 512], ps[:],
                            Act.Identity, bias=BIAS[:, li : li + 1], scale=1.0,
                        )

            def pool_slice(h, base):
                # Pool vv=v/2 chain over chunks [base, NCHH) of every batch in
                # half h; starts while DVE is still on the previous scan.
                c0 = h * NCHH + base
                AT4 = AT4S[h]
                W = NCHH - base
                for j in range(H + LC):
                    jj = (LC - H) + j
                    cs, l = jj // LC, jj % LC
                    col = XH4[:, :, c0 + cs : c0 + cs + W, l]
                    dst = SCRP[:, :, 0:W] if j < H else AT4[:, :, base:NCHH, j - H]
                    if j == 0:
                        nc.gpsimd.tensor_scalar(
                            dst, col, 1.0, 0.0, op0=Alu.mult, op1=Alu.add)
                    else:
                        nc.gpsimd.tensor_tensor(dst, VP[:, :, 0:W], col, op=Alu.add)
                    if j < H + LC - 1:
                        nc.gpsimd.tensor_scalar(
                            MP[:, :, 0:W], dst, float(VTH), 0.5,
                            op0=Alu.is_lt, op1=Alu.mult)
                        nc.gpsimd.tensor_tensor(
                            VP[:, :, 0:W], MP[:, :, 0:W], dst, op=Alu.mult)

            def scan(h, nchunks=NCHH, cbase=0, warm=H):
                c0 = h * NCHH + cbase
                AT4 = AT4S[h]
                nc2 = nchunks // 2
                subs = ((0, VA, SCRA), (nc2, VB, SCRB))
                NC2l = nc2
                for j in range(warm + LC):
                    jj = (LC - warm) + j
                    cs, l = jj // LC, jj % LC
                    cols, dsts = [], []
                    for coff, V, SCR in subs:
                        cols.append(XH4[:, :, c0 + coff + cs : c0 + coff + cs + NC2l, l])
                        dsts.append(SCR[:, :, 0:NC2l] if j < warm
                                    else AT4[:, :, cbase + coff : cbase + coff + NC2l, j - warm])
                    # A = 0.5*v + xh  (sub-chains interleaved to hide RAW latency)
                    for (coff, V, SCR), col, dst in zip(subs, cols, dsts):
                        if j == 0:
                            nc.vector.scalar_tensor_tensor(
                                dst, col, 0.0, col, op0=Alu.mult, op1=Alu.add)
                        else:
                            nc.vector.scalar_tensor_tensor(
                                dst, V[:, :, 0:NC2l], 0.5, col, op0=Alu.mult, op1=Alu.add)
                    # v' = (A < 1) * A
                    if j < warm + LC - 1:
                        for (coff, V, SCR), dst in zip(subs, dsts):
                            nc.vector.scalar_tensor_tensor(
                                V[:, :, 0:NC2l], dst, float(VTH), dst,
                                op0=Alu.is_lt, op1=Alu.mult)

            def spike_res(li, h):
                # Per-batch ops so conv(li+1)/out-DMA of batch b start as soon
                # as batch b's X is updated (batch order matches conv order).
                AT = ATS[h]
                final = li == len(DILATIONS) - 1 and h == NH - 1
                for b in range(BL):
                    xs = X[:, b, PADX + h * TH : PADX + (h + 1) * TH]
                    if final:
                        # half-batch granularity: each output DMA starts ~1us
                        # after its half's spikes land; last batch on Pool
                        # (idle by then) to shorten the serial DVE chain
                        T2 = TH // 2
                        for g in range(2):
                            xg = X[:, b, PADX + h * TH + g * T2 : PADX + h * TH + (g + 1) * T2]
                            ag = AT[:, b, g * T2 : (g + 1) * T2]
                            if b == BL - 1:
                                nc.gpsimd.tensor_scalar(
                                    ag, ag, float(VTH), 1.0, op0=Alu.is_ge, op1=Alu.mult)
                                nc.gpsimd.tensor_tensor(xg, ag, xg, op=Alu.add)
                            else:
                                nc.vector.scalar_tensor_tensor(
                                    xg, ag, float(VTH), xg, op0=Alu.is_ge, op1=Alu.add)
                    else:
                        # Pool: s = (A >= 1) overwrites AT in place, then
                        # X += s; half-batch granularity so the next layer's
                        # conv tiles unblock earlier
                        T2 = TH // 2
                        for g in range(2):
                            xg = X[:, b, PADX + h * TH + g * T2 : PADX + h * TH + (g + 1) * T2]
                            ag = AT[:, b, g * T2 : (g + 1) * T2]
                            nc.gpsimd.tensor_scalar(
                                ag, ag, float(VTH), 1.0, op0=Alu.is_ge, op1=Alu.mult)
                            nc.gpsimd.tensor_tensor(xg, ag, xg, op=Alu.add)

            # Issue order: both halves' convs precede the layer's sr ops so the
            # h=1 conv halo reads pre-spike X (correctness) and PE pipelines
            # ahead of the scans (conv(li,1) runs during scan(li,0), and
            # conv(li+1,0) during scan(li,1)).
            LAST = len(DILATIONS) - 1
            for li, d in enumerate(DILATIONS):
                conv(li, d, 0, f32r_layers[li])
                conv(li, d, 1, f32r_layers[li])
                for h in range(NH):
                    # layer 4's warmup error cannot cascade (no layers after)
                    scan(h, warm=10 if li == LAST else H)
                    spike_res(li, h)
                    if li == LAST:
                        for b in range(BL):
                            if h == NH - 1:
                                # final half: half-batch DMAs on alternating
                                # queues, following the finer sr ops
                                T2 = TH // 2
                                for g in range(2):
                                    q = (nc.sync, nc.scalar)[(2 * b + g) % 2]
                                    q.dma_start(
                                        o_d[b][:, h * TH + g * T2 : h * TH + (g + 1) * T2],
                                        X[:, b, PADX + h * TH + g * T2 : PADX + h * TH + (g + 1) * T2],
                                    )
                            else:
                                nc.sync.dma_start(
                                    o_d[b][:, h * TH : (h + 1) * TH],
                                    X[:, b, PADX + h * TH : PADX + (h + 1) * TH],
                                )

    nc.compile()
    return nc


def kernel(x, w, gamma, beta, mean, var, **_):
    from concourse.bass_utils import run_bass_kernel_spmd

    x = np.ascontiguousarray(x, np.float32)
    inv = (gamma / np.sqrt(var + EPS)).astype(np.float32)          # [4, C]
    # wt[ci, l, k, co] = 0.5 * w[l, co, ci, k] * inv[l, co]
    wt = (0.5 * w * inv[:, :, None, None]).astype(np.float32)      # [4, Co, Ci, K]
    wt = np.ascontiguousarray(wt.transpose(2, 0, 3, 1))            # [Ci, 4, K, Co]
    bias = (0.5 * (beta - mean * inv)).astype(np.float32).T        # [C, 4]
    bias = np.ascontiguousarray(bias)

    if "nc" not in _cache:
        _cache["nc"] = _build()
    nc = _cache["nc"]

    extra = {}
    if any(F32R_LAYER):
        extra["zpad"] = np.zeros((C, BL * PADX), np.float32)
    in_maps = [
        {"x": np.ascontiguousarray(x[i * BL : (i + 1) * BL]), "wt": wt, "bias": bias,
         **extra}
        for i in range(NCORES)
    ]
    res = run_bass_kernel_spmd(nc, in_maps, list(range(NCORES)))
    return np.concatenate([res.results[i]["out"] for i in range(NCORES)], axis=0)


# revision 7
# speedup vs baseline: 1.0452x; 1.0074x over previous
"""Trainium2 Bass kernel v2 for LongRangeTCN.

Per core (BL=4 batches), per layer, per T-half (TH=2048):
  conv: 3-tap dilated conv as PSUM-accumulated matmuls (per-layer fp32 or f32r),
        Act engine evacuates PSUM->XH adding folded BN bias (xh = 0.5*BN(conv)).
  scan: LIF wavefront on DVE over chunks of LC=32 with H=12 warmup
        (2 scalar_tensor_tensor ops per step, all 4 batches = 256 cols wide).
        The A-trajectory goes to a separate AT tile so XH stays read-only.
  sr:   spike + residual X += (A >= 1) on Pool/GpSimd (TS extract + TT add; the
        95ns Q7 launch amortizes over full-width ops). X is float32r so
        f32r-layer matmuls accept it; fp32 layers read a bitcast-fp32 view.
Pipeline: per-engine program order makes conv(li,h=1) on PE run during
scan(li,h=0) on DVE, Pool's sr(li,h) during scan(li,h+1).
"""

import numpy as np

TAU, VTH, EPS, K = 2.0, 1.0, 1e-5, 3
DILATIONS = (1, 2, 4, 8)
B, C, T = 32, 128, 4096
NCORES = 8
BL = B // NCORES          # 4 batches per core
LC = 32                   # scan chunk length
H = 12                    # warmup steps (0.5^H carry error ~2.4e-4)
NH = 2                    # T halves
TH = T // NH              # 2048
NCHH = TH // LC           # 64 chunks per batch per half
PADX = 16                 # conv left halo (max (K-1)*d)
SX = PADX + T             # 4112
PADH = LC                 # XH head zeros (warmup reads cols [LC-H, LC))
SXH = PADH + T            # 4128
# conv dtype per layer: True = f32r (1 cy/row, ~12-bit inputs), False = fp32
F32R_LAYER = (False, False, False, False)

_cache = {}


def _build(f32r_layers=F32R_LAYER):
    import concourse.bass as bass
    import concourse.bacc as bacc
    import concourse.tile as tile
    import concourse.mybir as mybir

    dt = mybir.dt.float32
    dtr = mybir.dt.float32r
    Alu = mybir.AluOpType
    Act = mybir.ActivationFunctionType

    any_f32r = any(f32r_layers)
    dtx = dtr if any_f32r else dt  # X/weights dtype: f32r only when needed
    nc = bacc.Bacc("TRN2", target_bir_lowering=False, debug=False)
    x_d = nc.dram_tensor("x", [BL, C, T], dtx, kind="ExternalInput")
    wt_d = nc.dram_tensor("wt", [C, 4, K, C], dtx, kind="ExternalInput")
    b_d = nc.dram_tensor("bias", [C, 4], dt, kind="ExternalInput")
    if any_f32r:
        z_d = nc.dram_tensor("zpad", [C, BL * PADX], dtx, kind="ExternalInput")
    o_d = nc.dram_tensor("out", [BL, C, T], dtx, kind="ExternalOutput")

    with tile.TileContext(nc) as tc:
        with (
            tc.tile_pool(name="big", bufs=1) as big,
            tc.tile_pool(name="small", bufs=1) as small,
            tc.tile_pool(name="psum", bufs=4, space="PSUM") as pp,
        ):
            X = big.tile([C, BL, SX], dtx, tag="X")
            XH = big.tile([C, BL, SXH], dt, tag="XH")
            ATS = [big.tile([C, BL, TH], dt, name=f"AT{h}", tag=f"AT{h}") for h in range(NH)]
            WT = small.tile([C, 4, K, C], dtx, tag="WT")
            BIAS = small.tile([C, 4], dt, tag="BIAS")
            # two independent sub-chains (A: chunks [0,NC2), B: [NC2,NCHH)) so
            # consecutive DVE ops never chain RAW back-to-back
            NC2 = NCHH // 2
            VA = small.tile([C, BL, NC2], dt, tag="VA")
            VB = small.tile([C, BL, NC2], dt, tag="VB")
            SCRA = small.tile([C, BL, NC2], dt, tag="SCRA")
            SCRB = small.tile([C, BL, NC2], dt, tag="SCRB")
            # Pool tail-scan slice state (final layer h=1): vv = v/2 chain
            CP2 = 40  # chunks/batch handled by Pool in the final scan
            VP = small.tile([C, BL, CP2], dt, tag="VP")
            MP = small.tile([C, BL, CP2], dt, tag="MP")
            SCRP = small.tile([C, BL, CP2], dt, tag="SCRP")

            # layer-1 weights first so the first conv isn't queued behind the
            # full weight load
            nc.sync.dma_start(WT[:, 0], wt_d[:, 0])
            nc.sync.dma_start(BIAS[:], b_d[:])
            if any_f32r:
                for b in range(BL):
                    nc.sync.dma_start(X[:, b, 0:PADX], z_d[:, b * PADX : (b + 1) * PADX])
            else:
                nc.vector.memset(X[:, :, 0:PADX], 0.0)
            nc.vector.memset(XH[:, :, 0:PADH], 0.0)
            for b in range(BL):
                for h in range(NH):
                    for g in range(2):
                        T2 = TH // 2
                        o0 = h * TH + g * T2
                        nc.sync.dma_start(
                            X[:, b, PADX + o0 : PADX + o0 + T2],
                            x_d[b][:, o0 : o0 + T2],
                        )
            nc.sync.dma_start(WT[:, 1:4], wt_d[:, 1:4])

            XH4 = XH[:].rearrange("p a (c l) -> p a c l", l=LC)   # c: 129
            AT4S = [A[:].rearrange("p a (c l) -> p a c l", l=LC) for A in ATS]
            Xf = X[:].bitcast(dt)  # exact-bits view for fp32 layers

            def conv(li, d, h, use_f32r, thalf_major=False):
                # b-major order coalesces best in steady state; the final
                # conv uses half-T-major so the last scan's first half-
                # wavefront starts ~20us before the last evacs land.
                if thalf_major:
                    order = [(th2 * 2 + tt, b) for th2 in range(2)
                             for b in range(BL) for tt in range(2)]
                else:
                    order = [(tt, b) for b in range(BL) for tt in range(TH // 512)]
                for tt, b in order:
                    if True:
                        t0 = h * TH + tt * 512
                        ps = pp.tile([C, 512], dt, tag="ps")
                        for k in range(K):
                            sh = (K - 1 - k) * d
                            if use_f32r:
                                rhs = X[:, b, PADX + t0 - sh : PADX + t0 - sh + 512]
                                lhsT = WT[:, li, k, :]
                            else:
                                rhs = Xf[:, b, PADX + t0 - sh : PADX + t0 - sh + 512]
                                lhsT = WT[:, li, k, :].bitcast(dt)
                            nc.tensor.matmul(
                                ps[:], lhsT, rhs, start=(k == 0), stop=(k == K - 1)
                            )
                        nc.scalar.activation(
                            XH[:, b, PADH + t0 : PADH + t0 + 512], ps[:],
                            Act.Identity, bias=BIAS[:, li : li + 1], scale=1.0,
                        )

            def pool_slice(h, base):
                # Pool vv=v/2 chain over chunks [base, NCHH) of every batch in
                # half h; starts while DVE is still on the previous scan.
                c0 = h * NCHH + base
                AT4 = AT4S[h]
                W = NCHH - base
                for j in range(H + LC):
                    jj = (LC - H) + j
                    cs, l = jj // LC, jj % LC
                    col = XH4[:, :, c0 + cs : c0 + cs + W, l]
                    dst = SCRP[:, :, 0:W] if j < H else AT4[:, :, base:NCHH, j - H]
                    if j == 0:
                        nc.gpsimd.tensor_scalar(
                            dst, col, 1.0, 0.0, op0=Alu.mult, op1=Alu.add)
                    else:
                        nc.gpsimd.tensor_tensor(dst, VP[:, :, 0:W], col, op=Alu.add)
                    if j < H + LC - 1:
                        nc.gpsimd.tensor_scalar(
                            MP[:, :, 0:W], dst, float(VTH), 0.5,
                            op0=Alu.is_lt, op1=Alu.mult)
                        nc.gpsimd.tensor_tensor(
                            VP[:, :, 0:W], MP[:, :, 0:W], dst, op=Alu.mult)

            def scan(h, nchunks=NCHH, cbase=0, warm=H):
                c0 = h * NCHH + cbase
                AT4 = AT4S[h]
                nc2 = nchunks // 2
                subs = ((0, VA, SCRA), (nc2, VB, SCRB))
                NC2l = nc2
                for j in range(warm + LC):
                    jj = (LC - warm) + j
                    cs, l = jj // LC, jj % LC
                    cols, dsts = [], []
                    for coff, V, SCR in subs:
                        cols.append(XH4[:, :, c0 + coff + cs : c0 + coff + cs + NC2l, l])
                        dsts.append(SCR[:, :, 0:NC2l] if j < warm
                                    else AT4[:, :, cbase + coff : cbase + coff + NC2l, j - warm])
                    # A = 0.5*v + xh  (sub-chains interleaved to hide RAW latency)
                    for (coff, V, SCR), col, dst in zip(subs, cols, dsts):
                        if j == 0:
                            nc.vector.scalar_tensor_tensor(
                                dst, col, 0.0, col, op0=Alu.mult, op1=Alu.add)
                        else:
                            nc.vector.scalar_tensor_tensor(
                                dst, V[:, :, 0:NC2l], 0.5, col, op0=Alu.mult, op1=Alu.add)
                    # v' = (A < 1) * A
                    if j < warm + LC - 1:
                        for (coff, V, SCR), dst in zip(subs, dsts):
                            nc.vector.scalar_tensor_tensor(
                                V[:, :, 0:NC2l], dst, float(VTH), dst,
                                op0=Alu.is_lt, op1=Alu.mult)

            def spike_res(li, h):
                # Per-batch ops so conv(li+1)/out-DMA of batch b start as soon
                # as batch b's X is updated (batch order matches conv order).
                AT = ATS[h]
                final = li == len(DILATIONS) - 1 and h == NH - 1
                for b in range(BL):
                    xs = X[:, b, PADX + h * TH : PADX + (h + 1) * TH]
                    if final:
                        # half-batch granularity: each output DMA starts ~1us
                        # after its half's spikes land; last batch on Pool
                        # (idle by then) to shorten the serial DVE chain
                        T2 = TH // 2
                        for g in range(2):
                            xg = X[:, b, PADX + h * TH + g * T2 : PADX + h * TH + (g + 1) * T2]
                            ag = AT[:, b, g * T2 : (g + 1) * T2]
                            if b == BL - 1:
                                nc.gpsimd.tensor_scalar(
                                    ag, ag, float(VTH), 1.0, op0=Alu.is_ge, op1=Alu.mult)
                                nc.gpsimd.tensor_tensor(xg, ag, xg, op=Alu.add)
                            else:
                                nc.vector.scalar_tensor_tensor(
                                    xg, ag, float(VTH), xg, op0=Alu.is_ge, op1=Alu.add)
                    else:
                        # Pool: s = (A >= 1) overwrites AT in place, then
                        # X += s; half-batch granularity so the next layer's
                        # conv tiles unblock earlier
                        T2 = TH // 2
                        for g in range(2):
                            xg = X[:, b, PADX + h * TH + g * T2 : PADX + h * TH + (g + 1) * T2]
                            ag = AT[:, b, g * T2 : (g + 1) * T2]
                            nc.gpsimd.tensor_scalar(
                                ag, ag, float(VTH), 1.0, op0=Alu.is_ge, op1=Alu.mult)
                            nc.gpsimd.tensor_tensor(xg, ag, xg, op=Alu.add)

            # Issue order: both halves' convs precede the layer's sr ops so the
            # h=1 conv halo reads pre-spike X (correctness) and PE pipelines
            # ahead of the scans (conv(li,1) runs during scan(li,0), and
            # conv(li+1,0) during scan(li,1)).
            LAST = len(DILATIONS) - 1
            for li, d in enumerate(DILATIONS):
                conv(li, d, 0, f32r_layers[li])
                conv(li, d, 1, f32r_layers[li])
                for h in range(NH):
                    # layer 4's warmup error cannot cascade (no layers after)
                    scan(h, warm=10 if li == LAST else H)
                    spike_res(li, h)
                    if li == LAST:
                        for b in range(BL):
                            if h == NH - 1:
                                # final half: half-batch DMAs on alternating
                                # queues, following the finer sr ops
                                T2 = TH // 2
                                for g in range(2):
                                    q = (nc.sync, nc.scalar)[(2 * b + g) % 2]
                                    q.dma_start(
                                        o_d[b][:, h * TH + g * T2 : h * TH + (g + 1) * T2],
                                        X[:, b, PADX + h * TH + g * T2 : PADX + h * TH + (g + 1) * T2],
                                    )
                            else:
                                nc.sync.dma_start(
                                    o_d[b][:, h * TH : (h + 1) * TH],
                                    X[:, b, PADX + h * TH : PADX + (h + 1) * TH],
                                )

    nc.compile()
    return nc


def kernel(x, w, gamma, beta, mean, var, **_):
    from concourse.bass_utils import run_bass_kernel_spmd

    x = np.ascontiguousarray(x, np.float32)
    inv = (gamma / np.sqrt(var + EPS)).astype(np.float32)          # [4, C]
    # wt[ci, l, k, co] = 0.5 * w[l, co, ci, k] * inv[l, co]
    wt = (0.5 * w * inv[:, :, None, None]).astype(np.float32)      # [4, Co, Ci, K]
    wt = np.ascontiguousarray(wt.transpose(2, 0, 3, 1))            # [Ci, 4, K, Co]
    bias = (0.5 * (beta - mean * inv)).astype(np.float32).T        # [C, 4]
    bias = np.ascontiguousarray(bias)

    if "nc" not in _cache:
        _cache["nc"] = _build()
    nc = _cache["nc"]

    extra = {}
    if any(F32R_LAYER):
        extra["zpad"] = np.zeros((C, BL * PADX), np.float32)
    in_maps = [
        {"x": np.ascontiguousarray(x[i * BL : (i + 1) * BL]), "wt": wt, "bias": bias,
         **extra}
        for i in range(NCORES)
    ]
    res = run_bass_kernel_spmd(nc, in_maps, list(range(NCORES)))
    return np.concatenate([res.results[i]["out"] for i in range(NCORES)], axis=0)
